# revision 8
# baseline (speedup 1.0000x reference)
"""Causal attention (single head) on 8 Trainium2 NeuronCores — v3.

Problem: x[4096,1024], Wq/Wk/Wv[1024,1024] (torch Linear layout, applied as
x @ W.T); out = renormalized-causal-softmax(Q K^T / 32) @ V, fp32, [4096,1024].

Distribution (hardcoded for S=4096, D=1024, 8 cores):
  - Q rows sharded STRIDED: core c owns rows c::8; with 128-row q-tiles, tile
    qt spans global rows [1024qt, 1024qt+1024) so all cores share one program
    (SPMD), and the intra-tile causal mask is a per-core input tensor.
  - K/V rows sharded CONTIGUOUS: core c computes K,V for rows [512c, 512c+512),
    exchanged via two AllGathers: K^T in fp8e4 (x16 scale, 4MB out) and V in
    fp8e3 (x2 scale, 4MB out).  CC facts measured on this fabric: a one-time
    ~25us bootstrap barrier ends ~45us regardless of kernel structure, the
    first AG starts ~11us after it, subsequent AGs ~1.8us apart, each 4MB AG
    moves in ~23.5us.  More/smaller CC ops lose (~10us fixed per op), so
    exactly two payload AGs, no dummy ops.
  - Scores are computed TRANSPOSED (S^T[k,q] tiles): no P-transposes, and the
    softmax denominator is a third matmul (rhs=ones[128,1]) reusing the P^T
    stationary already loaded for P@V.
  - Score matmuls: fp8 DoubleRow with lhsT = K chunk duplicated into both
    k-tile slots (stride-0 AP) and rhs = (Q_hi, Q_lo), an e4m3 hi/lo split of
    16*Q.  Cycle-neutral vs bf16 (the 2nd slot is spent on Q_lo), but Q costs
    no precision, which is what lets V ship as e3m4 (Q-e4m3 + V-e3m4 together
    would break the 2e-2 error budget; this scheme measures 1.68e-2).
  - exp needs no max-subtraction (scores within exp range at this scale); the
    causal mask is an additive -3e7 on the pre-scale psum (data, not code).
    The reference's "softmax -> tril -> renormalize" is algebraically
    identical to masked exp / masked sum.
  - P@V: lhsT = P^T bf16 direct from exp; rhs = V e3m4 (subnormals verified
    to work); both d-halves + denominator share one LDWEIGHTS per P^T chunk;
    fp32 PSUM accumulate; output scaled by 1/(2*den).
"""

import numpy as np
import ml_dtypes

S, D, NC_N = 4096, 1024, 8
QROWS = S // NC_N            # 512 q rows per core
KVROWS = S // NC_N           # 512 kv rows per core
NQT = QROWS // 128           # 4 q-tiles of 128 rows per core
DC = D // 128                # 8 contraction chunks
NKC = S // 128               # 32 key chunks of 128 rows
BF16 = ml_dtypes.bfloat16

SC_Q = 16.0                  # Q pre-scale before e4m3 hi/lo split
SC_K = 16.0                  # K pre-scale before e4m3
SC_V = 2.0                   # V pre-scale before e3m4
MASKNEG = -3.0e7             # additive causal mask on pre-scale psum

_CACHE = {}


def _build():
    import concourse.bass as bass
    import concourse.mybir as mybir
    import concourse.tile as tile
    from concourse import bacc

    fp32 = mybir.dt.float32
    bf16 = mybir.dt.bfloat16
    fp8 = mybir.dt.float8e4
    fp8e3 = mybir.dt.float8e3
    DR = mybir.MatmulPerfMode.DoubleRow
    MUL = mybir.AluOpType.mult
    SUB = mybir.AluOpType.subtract

    nc = bacc.Bacc("TRN2", target_bir_lowering=False, debug=False,
                   num_devices=NC_N, enable_asserts=False)

    xt_q = nc.dram_tensor("xt_q", [D, QROWS], bf16, kind="ExternalInput").ap()
    xt_kv = nc.dram_tensor("xt_kv", [D, KVROWS], bf16, kind="ExternalInput").ap()
    wqt = nc.dram_tensor("wqt", [D, D], bf16, kind="ExternalInput").ap()
    wkt = nc.dram_tensor("wkt", [D, D], bf16, kind="ExternalInput").ap()
    wvt = nc.dram_tensor("wvt", [D, D], bf16, kind="ExternalInput").ap()
    maskt = nc.dram_tensor("maskt", [128, 1024], fp32, kind="ExternalInput").ap()
    out = nc.dram_tensor("out", [QROWS, D], bf16, kind="ExternalOutput").ap()

    rg = [list(range(NC_N))]
    act_scale = 1.0 / (np.sqrt(np.float32(D)) * SC_Q * SC_K)

    with tile.TileContext(nc) as tc:
        with (
            tc.tile_pool(name="dram", bufs=1, space="DRAM") as dram,
            tc.tile_pool(name="const", bufs=1) as cpool,
            tc.tile_pool(name="kvres", bufs=1) as kvpool,
            tc.tile_pool(name="stats", bufs=4) as stpool,
        ):
            kt_cc_in = dram.tile([D, KVROWS], fp8, name="kt_cc_in")
            kt_cc_out = dram.tile([NC_N, D, KVROWS], fp8, name="kt_cc_out",
                                  addr_space="Shared")
            # declared fp8e4 and [D, KVROWS]-shaped exactly like kt_cc
            # (fp8e3-typed and [KVROWS, D]-shaped AllGathers both measured
            # ~2x slower per byte); the bytes are e3m4 V rows and the DMA APs
            # bitcast/reshape accordingly.  Linear layout: byte offset of
            # V[s, d] is s*1024 + d = row (2s + d//512), col (d%512).
            v_cc_in = dram.tile([D, KVROWS], fp8, name="v_cc_in")
            v_cc_out = dram.tile([NC_N, D, KVROWS], fp8, name="v_cc_out",
                                 addr_space="Shared")

            ones = cpool.tile([128, 1], bf16, name="ones")
            nc.gpsimd.memset(ones[:], 1.0)
            maskt_sb = cpool.tile([128, 1024], fp32, name="maskt_sb")

            # gathered K^T: ktf[r][p, dc*512+j] = 16*K[512r+j, 128dc+p]
            ktf = [kvpool.tile([128, DC * 512], fp8, name=f"ktf{r}")
                   for r in range(NC_N)]
            # gathered V: vf[r][p, sl*1024 + j] = 2*V[512r+128sl+p, j]
            vf = [kvpool.tile([128, 4 * 1024], fp8e3, name=f"vf{r}")
                  for r in range(NC_N)]
            # Q hi/lo packed: qthl[p, dc*1024 + hl*512 + q] = e4m3 hi/lo of
            # 16*Q[q_local, 128dc+p]
            qthl = kvpool.tile([128, DC * 1024], fp8, name="qthl")
            # P^T: pt[p, kc*512 + q] = exp(S^T)[128kc+p, q] (cols q>=qoff valid)
            pt = kvpool.tile([128, NKC * 512], bf16, name="pt")

            # ---------------- phase 1: projections + gathers ----------------
            with (
                tc.tile_pool(name="wpool", bufs=1) as wpool,
                tc.tile_pool(name="xpool", bufs=1) as xpool,
                tc.tile_pool(name="loc", bufs=1) as locpool,
                tc.tile_pool(name="ppsum", bufs=6, space="PSUM") as ppsum,
                tc.tile_pool(name="wpsum", bufs=1, space="PSUM") as wpsum,
            ):
                # big merged loads; K-projection inputs first (CC critical path)
                wk = wpool.tile([128, DC * D], bf16, name="wk")
                xkv = xpool.tile([128, DC * KVROWS], bf16, name="xkv")
                for dc in range(DC):
                    nc.sync.dma_start(xkv[:, dc * KVROWS:(dc + 1) * KVROWS],
                                      xt_kv[dc * 128:(dc + 1) * 128, :])
                    nc.scalar.dma_start(wk[:, dc * D:(dc + 1) * D],
                                        wkt[dc * 128:(dc + 1) * 128, :])
                wv = wpool.tile([128, DC * D], bf16, name="wv")
                nc.scalar.dma_start(
                    wv[:].rearrange("p (a j) -> p a j", a=DC),
                    wvt[:].rearrange("(a p) j -> p a j", p=128))
                xq = xpool.tile([128, DC * QROWS], bf16, name="xq")
                nc.sync.dma_start(
                    xq[:].rearrange("p (a j) -> p a j", a=DC),
                    xt_q[:].rearrange("(a p) j -> p a j", p=128))
                wq = wpool.tile([128, DC * D], bf16, name="wq")
                nc.scalar.dma_start(
                    wq[:].rearrange("p (a j) -> p a j", a=DC),
                    wqt[:].rearrange("(a p) j -> p a j", p=128))
                nc.scalar.dma_start(maskt_sb[:], maskt[:])

                # PE warmup: ~4us of throwaway matmuls on the first
                # loaded chunks so the HAM clock gate opens before K-proj
                # (stall-riddled cold starts pace at 1.2GHz vs 1.95GHz warm).
                wps = wpsum.tile([128, 512], fp32, name="warm_ps")
                for _ in range(16):
                    nc.tensor.matmul(wps[:], wk[:, 0:128], xkv[:, 0:512],
                                     start=True, stop=True)
                scrap = locpool.tile([128, 512], bf16, name="scrap")
                nc.vector.tensor_copy(scrap[:], wps[:])

                # K^T_local[d, s] = 16 * (Wk @ x_kv^T) -> e4m3, one CC write
                lock = locpool.tile([128, DC * 512], fp8, name="lock")
                for po in range(DC):
                    ps = ppsum.tile([128, 512], fp32, tag="pp")
                    for dc in range(DC):
                        nc.tensor.matmul(
                            ps[:], wk[:, dc * D + po * 128:dc * D + (po + 1) * 128],
                            xkv[:, dc * 512:(dc + 1) * 512],
                            start=(dc == 0), stop=(dc == DC - 1))
                    nc.vector.tensor_scalar_mul(
                        lock[:, po * 512:(po + 1) * 512], ps[:], SC_K)
                    nc.sync.dma_start(
                        kt_cc_in[po * 128:(po + 1) * 128, :],
                        lock[:, po * 512:(po + 1) * 512])

                nc.gpsimd.collective_compute(
                    "AllGather", mybir.AluOpType.bypass, replica_groups=rg,
                    ins=[kt_cc_in[:]], outs=[kt_cc_out[:]])

                # V_local[s, d] = 2 * (x_kv @ Wv^T) -> e3m4, one CC write
                locv = locpool.tile([128, 4 * D], fp8e3, name="locv")
                for st in range(4):
                    for dh in range(2):
                        ps = ppsum.tile([128, 512], fp32, tag="pp")
                        for dc in range(DC):
                            nc.tensor.matmul(
                                ps[:], xkv[:, dc * 512 + st * 128:
                                           dc * 512 + (st + 1) * 128],
                                wv[:, dc * D + dh * 512:dc * D + (dh + 1) * 512],
                                start=(dc == 0), stop=(dc == DC - 1))
                        lv = locv[:, st * D + dh * 512:st * D + (dh + 1) * 512]
                        nc.vector.tensor_scalar_mul(lv, ps[:], SC_V)
                        nc.sync.dma_start(
                            v_cc_in[:].bitcast(fp8e3)
                            .rearrange("(s two) c -> s two c", two=2)
                            [st * 128:(st + 1) * 128, dh, :],
                            lv)
                nc.gpsimd.collective_compute(
                    "AllGather", mybir.AluOpType.bypass, replica_groups=rg,
                    ins=[v_cc_in[:]], outs=[v_cc_out[:]])

                # Q-projection -> 16*Q -> e4m3 hi/lo packed into qthl
                for po in range(DC):
                    ps = ppsum.tile([128, 512], fp32, tag="pp")
                    for dc in range(DC):
                        nc.tensor.matmul(
                            ps[:], wq[:, dc * D + po * 128:dc * D + (po + 1) * 128],
                            xq[:, dc * 512:(dc + 1) * 512],
                            start=(dc == 0), stop=(dc == DC - 1))
                    qh = qthl[:, po * 1024:po * 1024 + 512]
                    ql = qthl[:, po * 1024 + 512:(po + 1) * 1024]
                    nc.vector.tensor_scalar_mul(qh, ps[:], SC_Q)
                    nc.vector.scalar_tensor_tensor(ql, ps[:], SC_Q, qh, MUL, SUB)

            # ---------------- phase 2: pull gathered K/V into SBUF ----------
            pull_engs = [nc.sync, nc.scalar, nc.gpsimd]
            for r in range(NC_N):
                eng = pull_engs[r % 3]
                eng.dma_start(
                    ktf[r][:].rearrange("p (a j) -> p a j", a=DC),
                    kt_cc_out[r].rearrange("(a p) j -> p a j", p=128))
            for r in range(NC_N):
                eng = pull_engs[(r + 1) % 3]
                eng.dma_start(
                    vf[r][:].rearrange("p (a b j) -> p a b j", a=4, b=2),
                    v_cc_out[r].bitcast(fp8e3)
                    .rearrange("(a p two) j -> p a two j", p=128, two=2))

            # ---------------- phase 3: scores transposed + exp --------------
            with (
                tc.tile_pool(name="spsum", bufs=3, space="PSUM") as spsum,
                tc.tile_pool(name="opsum", bufs=3, space="PSUM") as opsum,
                tc.tile_pool(name="dpsum", bufs=2, space="PSUM") as dpsum,
                tc.tile_pool(name="obuf", bufs=2) as opool,
            ):
                for kc in range(NKC):
                    r, sl = kc // 4, kc % 4
                    qoff = (kc // 8) * 128
                    w = 512 - qoff
                    ps = spsum.tile([128, 512], fp32, tag="s")
                    for dc in range(DC):
                        lhsT = (ktf[r][:, dc * 512 + sl * 128:
                                       dc * 512 + (sl + 1) * 128]
                                .unsqueeze(1).broadcast_to([128, 2, 128]))
                        rhs = (qthl[:, dc * 1024:(dc + 1) * 1024]
                               .rearrange("p (a j) -> p a j", a=2)[:, :, qoff:])
                        nc.tensor.matmul(ps[:, 0:w], lhsT, rhs, perf_mode=DR,
                                         start=(dc == 0), stop=(dc == DC - 1))
                    nc.vector.tensor_add(
                        ps[:, 0:128], ps[:, 0:128],
                        maskt_sb[:, (kc % 8) * 128:(kc % 8 + 1) * 128])
                    nc.scalar.activation(
                        pt[:, kc * 512 + qoff:(kc + 1) * 512], ps[:, 0:w],
                        mybir.ActivationFunctionType.Exp,
                        bias=0.0, scale=float(act_scale))

                # ---------------- phase 4: P @ V + denominator --------------
                for qt in range(NQT):
                    nkc = 8 * (qt + 1)
                    pso = [opsum.tile([128, 512], fp32, tag="po",
                                      name=f"pso{qt}_{dh}") for dh in range(2)]
                    pden = dpsum.tile([128, 1], fp32, tag="d",
                                      name=f"pden{qt}")
                    for kc in range(nkc):
                        r, sl = kc // 4, kc % 4
                        lhsT = pt[:, kc * 512 + qt * 128:
                                  kc * 512 + (qt + 1) * 128]
                        for dh in range(2):
                            nc.tensor.matmul(
                                pso[dh][:], lhsT,
                                vf[r][:, sl * 1024 + dh * 512:
                                      sl * 1024 + (dh + 1) * 512],
                                start=(kc == 0), stop=(kc == nkc - 1))
                        nc.tensor.matmul(
                            pden[:], lhsT, ones[:],
                            start=(kc == 0), stop=(kc == nkc - 1))
                    den2 = stpool.tile([128, 1], fp32, tag="den")
                    recip = stpool.tile([128, 1], fp32, tag="recip")
                    nc.vector.tensor_scalar_mul(den2[:], pden[:], SC_V)
                    nc.vector.reciprocal(recip[:], den2[:])
                    o_sb = opool.tile([128, D], bf16, tag="o")
                    for dh in range(2):
                        nc.vector.tensor_scalar_mul(
                            o_sb[:, dh * 512:(dh + 1) * 512], pso[dh][:],
                            recip[:])
                    nc.sync.dma_start(out[qt * 128:(qt + 1) * 128, :], o_sb[:])

    nc.compile()
    return nc


def _get_nc():
    if "nc" not in _CACHE:
        _CACHE["nc"] = _build()
    return _CACHE["nc"]


def make_in_maps(x, Wq, Wk, Wv):
    x_bf = np.ascontiguousarray(x).astype(BF16)
    wqt = np.ascontiguousarray(Wq.astype(BF16).T)
    wkt = np.ascontiguousarray(Wk.astype(BF16).T)
    wvt = np.ascontiguousarray(Wv.astype(BF16).T)
    in_maps = []
    for c in range(NC_N):
        xt_q = np.ascontiguousarray(x_bf[c::NC_N].T)
        xt_kv = np.ascontiguousarray(x_bf[c * KVROWS:(c + 1) * KVROWS].T)
        # maskt[p, 128a + i] = 0 if q >= k within the diagonal 1024-band:
        # q row i of a q-tile (global q = c + 8i + 1024qt), k row p of diag
        # chunk a (global k = 128a + p + 1024qt).
        p = np.arange(128)[:, None, None]
        a = np.arange(8)[None, :, None]
        i = np.arange(128)[None, None, :]
        keep = (c + 8 * i) >= (128 * a + p)
        maskt = np.where(keep, 0.0, MASKNEG).astype(np.float32).reshape(128, 1024)
        in_maps.append({"xt_q": xt_q, "xt_kv": xt_kv, "wqt": wqt,
                        "wkt": wkt, "wvt": wvt, "maskt": maskt})
    return in_maps


def run(in_maps, trace=False, tmpdir=None, trace_cores=None):
    from concourse.bass_utils import run_bass_kernel_spmd
    nc = _get_nc()
    return run_bass_kernel_spmd(nc, in_maps, core_ids=list(range(NC_N)),
                                trace=trace, tmpdir=tmpdir,
                                trace_cores=trace_cores)


def kernel(x, Wq, Wk, Wv):
    res = run(make_in_maps(np.asarray(x), np.asarray(Wq),
                           np.asarray(Wk), np.asarray(Wv)))
    full = np.empty((S, D), np.float32)
    for c in range(NC_N):
        full[c::NC_N] = res.results[c]["out"].astype(np.float32)
    return full


# revision 9
# speedup vs baseline: 1.0382x; 1.0382x over previous
"""Causal attention (single head) on 8 Trainium2 NeuronCores — v3.

Problem: x[4096,1024], Wq/Wk/Wv[1024,1024] (torch Linear layout, applied as
x @ W.T); out = renormalized-causal-softmax(Q K^T / 32) @ V, fp32, [4096,1024].

Distribution (hardcoded for S=4096, D=1024, 8 cores):
  - Q rows sharded STRIDED: core c owns rows c::8; with 128-row q-tiles, tile
    qt spans global rows [1024qt, 1024qt+1024) so all cores share one program
    (SPMD), and the intra-tile causal mask is a per-core input tensor.
  - K/V rows sharded CONTIGUOUS: core c computes K,V for rows [512c, 512c+512),
    exchanged via two AllGathers: K^T in fp8e4 (x16 scale, 4MB out) and V in
    fp8e3 (x2 scale, 4MB out).  CC facts measured on this fabric: a one-time
    ~25us bootstrap barrier ends ~45us regardless of kernel structure, the
    first AG starts ~11us after it, subsequent AGs ~1.8us apart, each 4MB AG
    moves in ~23.5us.  More/smaller CC ops lose (~10us fixed per op), so
    exactly two payload AGs, no dummy ops.
  - Scores are computed TRANSPOSED (S^T[k,q] tiles): no P-transposes, and the
    softmax denominator is a third matmul (rhs=ones[128,1]) reusing the P^T
    stationary already loaded for P@V.
  - Score matmuls: fp8 DoubleRow with lhsT = K chunk duplicated into both
    k-tile slots (stride-0 AP) and rhs = (Q_hi, Q_lo), an e4m3 hi/lo split of
    16*Q.  Cycle-neutral vs bf16 (the 2nd slot is spent on Q_lo), but Q costs
    no precision, which is what lets V ship as e3m4 (Q-e4m3 + V-e3m4 together
    would break the 2e-2 error budget; this scheme measures 1.68e-2).
  - exp needs no max-subtraction (scores within exp range at this scale); the
    causal mask is an additive -3e7 on the pre-scale psum (data, not code).
    The reference's "softmax -> tril -> renormalize" is algebraically
    identical to masked exp / masked sum.
  - P@V: lhsT = P^T bf16 direct from exp; rhs = V e3m4 (subnormals verified
    to work); both d-halves + denominator share one LDWEIGHTS per P^T chunk;
    fp32 PSUM accumulate; output scaled by 1/(2*den).
"""

import numpy as np
import ml_dtypes

S, D, NC_N = 4096, 1024, 8
QROWS = S // NC_N            # 512 q rows per core
KVROWS = S // NC_N           # 512 kv rows per core
NQT = QROWS // 128           # 4 q-tiles of 128 rows per core
DC = D // 128                # 8 contraction chunks
NKC = S // 128               # 32 key chunks of 128 rows
BF16 = ml_dtypes.bfloat16

SC_Q = 16.0                  # Q pre-scale before e4m3 hi/lo split
SC_K = 16.0                  # K pre-scale before e4m3
SC_V = 2.0                   # V pre-scale before e3m4
MASKNEG = -3.0e7             # additive causal mask on pre-scale psum

_CACHE = {}


def _build():
    import concourse.bass as bass
    import concourse.mybir as mybir
    import concourse.tile as tile
    from concourse import bacc

    fp32 = mybir.dt.float32
    bf16 = mybir.dt.bfloat16
    fp8 = mybir.dt.float8e4
    fp8e3 = mybir.dt.float8e3
    DR = mybir.MatmulPerfMode.DoubleRow
    MUL = mybir.AluOpType.mult
    SUB = mybir.AluOpType.subtract

    nc = bacc.Bacc("TRN2", target_bir_lowering=False, debug=False,
                   num_devices=NC_N, enable_asserts=False)

    xt_q = nc.dram_tensor("xt_q", [D, QROWS], bf16, kind="ExternalInput").ap()
    xt_kv = nc.dram_tensor("xt_kv", [D, KVROWS], bf16, kind="ExternalInput").ap()
    wqt = nc.dram_tensor("wqt", [D, D], bf16, kind="ExternalInput").ap()
    wkt = nc.dram_tensor("wkt", [D, D], bf16, kind="ExternalInput").ap()
    wvt = nc.dram_tensor("wvt", [D, D], bf16, kind="ExternalInput").ap()
    maskt = nc.dram_tensor("maskt", [128, 1024], fp32, kind="ExternalInput").ap()
    out = nc.dram_tensor("out", [QROWS, D], bf16, kind="ExternalOutput").ap()

    rg = [list(range(NC_N))]
    act_scale = 1.0 / (np.sqrt(np.float32(D)) * SC_Q * SC_K)

    with tile.TileContext(nc) as tc:
        with (
            tc.tile_pool(name="dram", bufs=1, space="DRAM") as dram,
            tc.tile_pool(name="const", bufs=1) as cpool,
            tc.tile_pool(name="kvres", bufs=1) as kvpool,
            tc.tile_pool(name="stats", bufs=4) as stpool,
        ):
            kt_cc_in = dram.tile([D, KVROWS], fp8, name="kt_cc_in")
            kt_cc_out = dram.tile([NC_N, D, KVROWS], fp8, name="kt_cc_out",
                                  addr_space="Shared")
            # declared fp8e4 and [D, KVROWS]-shaped exactly like kt_cc
            # (fp8e3-typed and [KVROWS, D]-shaped AllGathers both measured
            # ~2x slower per byte); the bytes are e3m4 V rows and the DMA APs
            # bitcast/reshape accordingly.  Linear layout: byte offset of
            # V[s, d] is s*1024 + d = row (2s + d//512), col (d%512).
            v_cc_in = dram.tile([D, KVROWS], fp8, name="v_cc_in")
            v_cc_out = dram.tile([NC_N, D, KVROWS], fp8, name="v_cc_out",
                                 addr_space="Shared")

            ones = cpool.tile([128, 1], bf16, name="ones")
            nc.gpsimd.memset(ones[:], 1.0)
            maskt_sb = cpool.tile([128, 1024], fp32, name="maskt_sb")

            # gathered K^T: ktf[r][p, dc*512+j] = 16*K[512r+j, 128dc+p]
            ktf = [kvpool.tile([128, DC * 512], fp8, name=f"ktf{r}")
                   for r in range(NC_N)]
            # gathered V: vf[r][p, sl*1024 + j] = 2*V[512r+128sl+p, j]
            vf = [kvpool.tile([128, 4 * 1024], fp8e3, name=f"vf{r}")
                  for r in range(NC_N)]
            # Q hi/lo packed: qthl[p, dc*1024 + hl*512 + q] = e4m3 hi/lo of
            # 16*Q[q_local, 128dc+p]
            qthl = kvpool.tile([128, DC * 1024], fp8, name="qthl")
            # P^T: pt[p, kc*512 + q] = exp(S^T)[128kc+p, q] (cols q>=qoff valid)
            pt = kvpool.tile([128, NKC * 512], bf16, name="pt")

            # ---------------- phase 1: projections + gathers ----------------
            with (
                tc.tile_pool(name="wpool", bufs=1) as wpool,
                tc.tile_pool(name="xpool", bufs=1) as xpool,
                tc.tile_pool(name="loc", bufs=1) as locpool,
                tc.tile_pool(name="ppsum", bufs=6, space="PSUM") as ppsum,
                tc.tile_pool(name="wpsum", bufs=1, space="PSUM") as wpsum,
            ):
                # big merged loads; K-projection inputs first (CC critical path)
                wk = wpool.tile([128, DC * D], bf16, name="wk")
                xkv = xpool.tile([128, DC * KVROWS], bf16, name="xkv")
                for dc in range(DC):
                    nc.sync.dma_start(xkv[:, dc * KVROWS:(dc + 1) * KVROWS],
                                      xt_kv[dc * 128:(dc + 1) * 128, :])
                    nc.scalar.dma_start(wk[:, dc * D:(dc + 1) * D],
                                        wkt[dc * 128:(dc + 1) * 128, :])
                wv = wpool.tile([128, DC * D], bf16, name="wv")
                nc.scalar.dma_start(
                    wv[:].rearrange("p (a j) -> p a j", a=DC),
                    wvt[:].rearrange("(a p) j -> p a j", p=128))
                wq = wpool.tile([128, DC * D], bf16, name="wq")
                xq = xpool.tile([128, DC * QROWS], bf16, name="xq")

                # PE warmup: ~4us of throwaway matmuls on the first
                # loaded chunks so the HAM clock gate opens before K-proj
                # (stall-riddled cold starts pace at 1.2GHz vs 1.95GHz warm).
                wps = wpsum.tile([128, 512], fp32, name="warm_ps")
                for _ in range(16):
                    nc.tensor.matmul(wps[:], wk[:, 0:128], xkv[:, 0:512],
                                     start=True, stop=True)
                scrap = locpool.tile([128, 512], bf16, name="scrap")
                nc.vector.tensor_copy(scrap[:], wps[:])

                # K^T_local[d, s] = 16 * (Wk @ x_kv^T) -> e4m3, one CC write
                lock = locpool.tile([128, DC * 512], fp8, name="lock")
                for po in range(DC):
                    ps = ppsum.tile([128, 512], fp32, tag="pp")
                    for dc in range(DC):
                        nc.tensor.matmul(
                            ps[:], wk[:, dc * D + po * 128:dc * D + (po + 1) * 128],
                            xkv[:, dc * 512:(dc + 1) * 512],
                            start=(dc == 0), stop=(dc == DC - 1))
                    nc.vector.tensor_scalar_mul(
                        lock[:, po * 512:(po + 1) * 512], ps[:], SC_K)
                    nc.sync.dma_start(
                        kt_cc_in[po * 128:(po + 1) * 128, :],
                        lock[:, po * 512:(po + 1) * 512])

                nc.gpsimd.collective_compute(
                    "AllGather", mybir.AluOpType.bypass, replica_groups=rg,
                    ins=[kt_cc_in[:]], outs=[kt_cc_out[:]])
                nc.sync.dma_start(
                    xq[:].rearrange("p (a j) -> p a j", a=DC),
                    xt_q[:].rearrange("(a p) j -> p a j", p=128))
                nc.scalar.dma_start(
                    wq[:].rearrange("p (a j) -> p a j", a=DC),
                    wqt[:].rearrange("(a p) j -> p a j", p=128))
                nc.scalar.dma_start(maskt_sb[:], maskt[:])

                # V_local[s, d] = 2 * (x_kv @ Wv^T) -> e3m4, one CC write
                locv = locpool.tile([128, 4 * D], fp8e3, name="locv")
                for st in range(4):
                    for dh in range(2):
                        ps = ppsum.tile([128, 512], fp32, tag="pp")
                        for dc in range(DC):
                            nc.tensor.matmul(
                                ps[:], xkv[:, dc * 512 + st * 128:
                                           dc * 512 + (st + 1) * 128],
                                wv[:, dc * D + dh * 512:dc * D + (dh + 1) * 512],
                                start=(dc == 0), stop=(dc == DC - 1))
                        lv = locv[:, st * D + dh * 512:st * D + (dh + 1) * 512]
                        nc.vector.tensor_scalar_mul(lv, ps[:], SC_V)
                        nc.sync.dma_start(
                            v_cc_in[:].bitcast(fp8e3)
                            .rearrange("(s two) c -> s two c", two=2)
                            [st * 128:(st + 1) * 128, dh, :],
                            lv)
                nc.gpsimd.collective_compute(
                    "AllGather", mybir.AluOpType.bypass, replica_groups=rg,
                    ins=[v_cc_in[:]], outs=[v_cc_out[:]])

                # Q-projection -> 16*Q -> e4m3 hi/lo packed into qthl
                for po in range(DC):
                    ps = ppsum.tile([128, 512], fp32, tag="pp")
                    for dc in range(DC):
                        nc.tensor.matmul(
                            ps[:], wq[:, dc * D + po * 128:dc * D + (po + 1) * 128],
                            xq[:, dc * 512:(dc + 1) * 512],
                            start=(dc == 0), stop=(dc == DC - 1))
                    qh = qthl[:, po * 1024:po * 1024 + 512]
                    ql = qthl[:, po * 1024 + 512:(po + 1) * 1024]
                    nc.vector.tensor_scalar_mul(qh, ps[:], SC_Q)
                    nc.vector.scalar_tensor_tensor(ql, ps[:], SC_Q, qh, MUL, SUB)

            # ---------------- phase 2: pull gathered K/V into SBUF ----------
            for r in range(NC_N):
                eng = nc.sync if r % 2 == 0 else nc.scalar
                eng.dma_start(
                    ktf[r][:].rearrange("p (a j) -> p a j", a=DC),
                    kt_cc_out[r].rearrange("(a p) j -> p a j", p=128))
            for r in range(NC_N):
                eng = nc.sync if r % 2 == 1 else nc.scalar
                eng.dma_start(
                    vf[r][:].rearrange("p (a b j) -> p a b j", a=4, b=2),
                    v_cc_out[r].bitcast(fp8e3)
                    .rearrange("(a p two) j -> p a two j", p=128, two=2))

            # ---------------- phase 3: scores transposed + exp --------------
            with (
                tc.tile_pool(name="spsum", bufs=3, space="PSUM") as spsum,
                tc.tile_pool(name="opsum", bufs=3, space="PSUM") as opsum,
                tc.tile_pool(name="dpsum", bufs=2, space="PSUM") as dpsum,
                tc.tile_pool(name="obuf", bufs=2) as opool,
            ):
                for kc in range(NKC):
                    r, sl = kc // 4, kc % 4
                    qoff = (kc // 8) * 128
                    w = 512 - qoff
                    ps = spsum.tile([128, 512], fp32, tag="s")
                    for dc in range(DC):
                        lhsT = (ktf[r][:, dc * 512 + sl * 128:
                                       dc * 512 + (sl + 1) * 128]
                                .unsqueeze(1).broadcast_to([128, 2, 128]))
                        rhs = (qthl[:, dc * 1024:(dc + 1) * 1024]
                               .rearrange("p (a j) -> p a j", a=2)[:, :, qoff:])
                        nc.tensor.matmul(ps[:, 0:w], lhsT, rhs, perf_mode=DR,
                                         start=(dc == 0), stop=(dc == DC - 1))
                    nc.vector.tensor_add(
                        ps[:, 0:128], ps[:, 0:128],
                        maskt_sb[:, (kc % 8) * 128:(kc % 8 + 1) * 128])
                    nc.scalar.activation(
                        pt[:, kc * 512 + qoff:(kc + 1) * 512], ps[:, 0:w],
                        mybir.ActivationFunctionType.Exp,
                        bias=0.0, scale=float(act_scale))

                # ---------------- phase 4: P @ V + denominator --------------
                for qt in range(NQT):
                    nkc = 8 * (qt + 1)
                    pso = [opsum.tile([128, 512], fp32, tag="po",
                                      name=f"pso{qt}_{dh}") for dh in range(2)]
                    pden = dpsum.tile([128, 1], fp32, tag="d",
                                      name=f"pden{qt}")
                    for kc in range(nkc):
                        r, sl = kc // 4, kc % 4
                        lhsT = pt[:, kc * 512 + qt * 128:
                                  kc * 512 + (qt + 1) * 128]
                        for dh in range(2):
                            nc.tensor.matmul(
                                pso[dh][:], lhsT,
                                vf[r][:, sl * 1024 + dh * 512:
                                      sl * 1024 + (dh + 1) * 512],
                                start=(kc == 0), stop=(kc == nkc - 1))
                        nc.tensor.matmul(
                            pden[:], lhsT, ones[:],
                            start=(kc == 0), stop=(kc == nkc - 1))
                    den2 = stpool.tile([128, 1], fp32, tag="den")
                    recip = stpool.tile([128, 1], fp32, tag="recip")
                    nc.vector.tensor_scalar_mul(den2[:], pden[:], SC_V)
                    nc.vector.reciprocal(recip[:], den2[:])
                    o_sb = opool.tile([128, D], bf16, tag="o")
                    for dh in range(2):
                        nc.vector.tensor_scalar_mul(
                            o_sb[:, dh * 512:(dh + 1) * 512], pso[dh][:],
                            recip[:])
                    nc.sync.dma_start(out[qt * 128:(qt + 1) * 128, :], o_sb[:])

    nc.compile()
    return nc


def _get_nc():
    if "nc" not in _CACHE:
        _CACHE["nc"] = _build()
    return _CACHE["nc"]


def make_in_maps(x, Wq, Wk, Wv):
    x_bf = np.ascontiguousarray(x).astype(BF16)
    wqt = np.ascontiguousarray(Wq.astype(BF16).T)
    wkt = np.ascontiguousarray(Wk.astype(BF16).T)
    wvt = np.ascontiguousarray(Wv.astype(BF16).T)
    in_maps = []
    for c in range(NC_N):
        xt_q = np.ascontiguousarray(x_bf[c::NC_N].T)
        xt_kv = np.ascontiguousarray(x_bf[c * KVROWS:(c + 1) * KVROWS].T)
        # maskt[p, 128a + i] = 0 if q >= k within the diagonal 1024-band:
        # q row i of a q-tile (global q = c + 8i + 1024qt), k row p of diag
        # chunk a (global k = 128a + p + 1024qt).
        p = np.arange(128)[:, None, None]
        a = np.arange(8)[None, :, None]
        i = np.arange(128)[None, None, :]
        keep = (c + 8 * i) >= (128 * a + p)
        maskt = np.where(keep, 0.0, MASKNEG).astype(np.float32).reshape(128, 1024)
        in_maps.append({"xt_q": xt_q, "xt_kv": xt_kv, "wqt": wqt,
                        "wkt": wkt, "wvt": wvt, "maskt": maskt})
    return in_maps


def run(in_maps, trace=False, tmpdir=None, trace_cores=None):
    from concourse.bass_utils import run_bass_kernel_spmd
    nc = _get_nc()
    return run_bass_kernel_spmd(nc, in_maps, core_ids=list(range(NC_N)),
                                trace=trace, tmpdir=tmpdir,
                                trace_cores=trace_cores)


def kernel(x, Wq, Wk, Wv):
    res = run(make_in_maps(np.asarray(x), np.asarray(Wq),
                           np.asarray(Wk), np.asarray(Wv)))
    full = np.empty((S, D), np.float32)
    for c in range(NC_N):
        full[c::NC_N] = res.results[c]["out"].astype(np.float32)
    return full


# revision 10
# speedup vs baseline: 1.1089x; 1.0681x over previous
"""Causal attention (single head) on 8 Trainium2 NeuronCores — v3.

Problem: x[4096,1024], Wq/Wk/Wv[1024,1024] (torch Linear layout, applied as
x @ W.T); out = renormalized-causal-softmax(Q K^T / 32) @ V, fp32, [4096,1024].

Distribution (hardcoded for S=4096, D=1024, 8 cores):
  - Q rows sharded STRIDED: core c owns rows c::8; with 128-row q-tiles, tile
    qt spans global rows [1024qt, 1024qt+1024) so all cores share one program
    (SPMD), and the intra-tile causal mask is a per-core input tensor.
  - K/V rows sharded CONTIGUOUS: core c computes K,V for rows [512c, 512c+512),
    exchanged via two AllGathers: K^T in fp8e4 (x16 scale, 4MB out) and V in
    fp8e3 (x2 scale, 4MB out).  CC facts measured on this fabric: a one-time
    ~25us bootstrap barrier ends ~45us regardless of kernel structure, the
    first AG starts ~11us after it, subsequent AGs ~1.8us apart, each 4MB AG
    moves in ~23.5us.  More/smaller CC ops lose (~10us fixed per op), so
    exactly two payload AGs, no dummy ops.
  - Scores are computed TRANSPOSED (S^T[k,q] tiles): no P-transposes, and the
    softmax denominator is a third matmul (rhs=ones[128,1]) reusing the P^T
    stationary already loaded for P@V.
  - Score matmuls: fp8 DoubleRow with lhsT = K chunk duplicated into both
    k-tile slots (stride-0 AP) and rhs = (Q_hi, Q_lo), an e4m3 hi/lo split of
    16*Q.  Cycle-neutral vs bf16 (the 2nd slot is spent on Q_lo), but Q costs
    no precision, which is what lets V ship as e3m4 (Q-e4m3 + V-e3m4 together
    would break the 2e-2 error budget; this scheme measures 1.68e-2).
  - exp needs no max-subtraction (scores within exp range at this scale); the
    causal mask is an additive -3e7 on the pre-scale psum (data, not code).
    The reference's "softmax -> tril -> renormalize" is algebraically
    identical to masked exp / masked sum.
  - P@V: lhsT = P^T bf16 direct from exp; rhs = V e3m4 (subnormals verified
    to work); both d-halves + denominator share one LDWEIGHTS per P^T chunk;
    fp32 PSUM accumulate; output scaled by 1/(2*den).
"""

import numpy as np
import ml_dtypes

S, D, NC_N = 4096, 1024, 8
QROWS = S // NC_N            # 512 q rows per core
KVROWS = S // NC_N           # 512 kv rows per core
NQT = QROWS // 128           # 4 q-tiles of 128 rows per core
DC = D // 128                # 8 contraction chunks
NKC = S // 128               # 32 key chunks of 128 rows
BF16 = ml_dtypes.bfloat16

SC_Q = 16.0                  # Q pre-scale before e4m3 hi/lo split
SC_K = 16.0                  # K pre-scale before e4m3
SC_V = 2.0                   # V pre-scale before e3m4
MASKNEG = -3.0e7             # additive causal mask on pre-scale psum

_CACHE = {}


def _build():
    import concourse.bass as bass
    import concourse.mybir as mybir
    import concourse.tile as tile
    from concourse import bacc

    fp32 = mybir.dt.float32
    bf16 = mybir.dt.bfloat16
    fp8 = mybir.dt.float8e4
    fp8e3 = mybir.dt.float8e3
    DR = mybir.MatmulPerfMode.DoubleRow
    MUL = mybir.AluOpType.mult
    SUB = mybir.AluOpType.subtract

    nc = bacc.Bacc("TRN2", target_bir_lowering=False, debug=False,
                   num_devices=NC_N, enable_asserts=False)

    xt_q = nc.dram_tensor("xt_q", [D, QROWS], bf16, kind="ExternalInput").ap()
    xt_kv = nc.dram_tensor("xt_kv", [D, KVROWS], bf16, kind="ExternalInput").ap()
    wqt = nc.dram_tensor("wqt", [D, D], bf16, kind="ExternalInput").ap()
    wkt = nc.dram_tensor("wkt", [D, D], bf16, kind="ExternalInput").ap()
    wvt = nc.dram_tensor("wvt", [D, D], bf16, kind="ExternalInput").ap()
    maskt = nc.dram_tensor("maskt", [128, 1024], fp32, kind="ExternalInput").ap()
    out = nc.dram_tensor("out", [QROWS, D], bf16, kind="ExternalOutput").ap()

    rg = [list(range(NC_N))]
    act_scale = 1.0 / (np.sqrt(np.float32(D)) * SC_Q * SC_K)

    with tile.TileContext(nc) as tc:
        with (
            tc.tile_pool(name="dram", bufs=1, space="DRAM") as dram,
            tc.tile_pool(name="const", bufs=1) as cpool,
            tc.tile_pool(name="kvres", bufs=1) as kvpool,
            tc.tile_pool(name="stats", bufs=4) as stpool,
        ):
            kt_cc_in = dram.tile([D, KVROWS], fp8, name="kt_cc_in")
            kt_cc_out = dram.tile([NC_N, D, KVROWS], fp8, name="kt_cc_out",
                                  addr_space="Shared")
            # declared fp8e4 and [D, KVROWS]-shaped exactly like kt_cc
            # (fp8e3-typed and [KVROWS, D]-shaped AllGathers both measured
            # ~2x slower per byte); the bytes are e3m4 V rows and the DMA APs
            # bitcast/reshape accordingly.  Linear layout: byte offset of
            # V[s, d] is s*1024 + d = row (2s + d//512), col (d%512).
            v_cc_in = dram.tile([D, KVROWS], fp8, name="v_cc_in")
            v_cc_out = dram.tile([NC_N, D, KVROWS], fp8, name="v_cc_out",
                                 addr_space="Shared")

            ones = cpool.tile([128, 1], bf16, name="ones")
            nc.gpsimd.memset(ones[:], 1.0)
            maskt_sb = cpool.tile([128, 1024], fp32, name="maskt_sb")

            # gathered K^T: ktf[r][p, dc*512+j] = 16*K[512r+j, 128dc+p]
            ktf = [kvpool.tile([128, DC * 512], fp8, name=f"ktf{r}")
                   for r in range(NC_N)]
            # gathered V: vf[r][p, sl*1024 + j] = 2*V[512r+128sl+p, j]
            vf = [kvpool.tile([128, 4 * 1024], fp8e3, name=f"vf{r}")
                  for r in range(NC_N)]
            # Q hi/lo packed: qthl[p, dc*1024 + hl*512 + q] = e4m3 hi/lo of
            # 16*Q[q_local, 128dc+p]
            qthl = kvpool.tile([128, DC * 1024], fp8, name="qthl")
            # P^T: pt[p, kc*512 + q] = exp(S^T)[128kc+p, q] (cols q>=qoff valid)
            pt = kvpool.tile([128, NKC * 512], bf16, name="pt")

            # ---------------- phase 1: projections + gathers ----------------
            with (
                tc.tile_pool(name="wpool", bufs=1) as wpool,
                tc.tile_pool(name="xpool", bufs=1) as xpool,
                tc.tile_pool(name="loc", bufs=1) as locpool,
                tc.tile_pool(name="ppsum", bufs=6, space="PSUM") as ppsum,
                tc.tile_pool(name="wpsum", bufs=1, space="PSUM") as wpsum,
            ):
                # big merged loads; K-projection inputs first (CC critical path)
                wk = wpool.tile([128, DC * D], bf16, name="wk")
                xkv = xpool.tile([128, DC * KVROWS], bf16, name="xkv")
                for dc in range(DC):
                    exv = nc.sync if dc % 2 == 0 else nc.scalar
                    exw = nc.scalar if dc % 2 == 0 else nc.sync
                    exv.dma_start(xkv[:, dc * KVROWS:(dc + 1) * KVROWS],
                                  xt_kv[dc * 128:(dc + 1) * 128, :])
                    exw.dma_start(wk[:, dc * D:(dc + 1) * D],
                                  wkt[dc * 128:(dc + 1) * 128, :])
                wv = wpool.tile([128, DC * D], bf16, name="wv")
                nc.scalar.dma_start(
                    wv[:].rearrange("p (a j) -> p a j", a=DC),
                    wvt[:].rearrange("(a p) j -> p a j", p=128))
                wq = wpool.tile([128, DC * D], bf16, name="wq")
                xq = xpool.tile([128, DC * QROWS], bf16, name="xq")

                # PE warmup: ~4us of throwaway matmuls on the first
                # loaded chunks so the HAM clock gate opens before K-proj
                # (stall-riddled cold starts pace at 1.2GHz vs 1.95GHz warm).
                wps = wpsum.tile([128, 512], fp32, name="warm_ps")
                for _ in range(16):
                    nc.tensor.matmul(wps[:], wk[:, 0:128], xkv[:, 0:512],
                                     start=True, stop=True)
                scrap = locpool.tile([128, 512], bf16, name="scrap")
                nc.vector.tensor_copy(scrap[:], wps[:])

                # K^T_local[d, s] = 16 * (Wk @ x_kv^T) -> e4m3, one CC write
                lock = locpool.tile([128, DC * 512], fp8, name="lock")
                for po in range(DC):
                    ps = ppsum.tile([128, 512], fp32, tag="pp")
                    for dc in range(DC):
                        nc.tensor.matmul(
                            ps[:], wk[:, dc * D + po * 128:dc * D + (po + 1) * 128],
                            xkv[:, dc * 512:(dc + 1) * 512],
                            start=(dc == 0), stop=(dc == DC - 1))
                    nc.vector.tensor_scalar_mul(
                        lock[:, po * 512:(po + 1) * 512], ps[:], SC_K)
                    nc.sync.dma_start(
                        kt_cc_in[po * 128:(po + 1) * 128, :],
                        lock[:, po * 512:(po + 1) * 512])

                nc.gpsimd.collective_compute(
                    "AllGather", mybir.AluOpType.bypass, replica_groups=rg,
                    ins=[kt_cc_in[:]], outs=[kt_cc_out[:]])
                nc.sync.dma_start(
                    xq[:].rearrange("p (a j) -> p a j", a=DC),
                    xt_q[:].rearrange("(a p) j -> p a j", p=128))
                nc.scalar.dma_start(
                    wq[:].rearrange("p (a j) -> p a j", a=DC),
                    wqt[:].rearrange("(a p) j -> p a j", p=128))
                nc.scalar.dma_start(maskt_sb[:], maskt[:])

                # V_local[s, d] = 2 * (x_kv @ Wv^T) -> e3m4, one CC write
                locv = locpool.tile([128, 4 * D], fp8e3, name="locv")
                for st in range(4):
                    for dh in range(2):
                        ps = ppsum.tile([128, 512], fp32, tag="pp")
                        for dc in range(DC):
                            nc.tensor.matmul(
                                ps[:], xkv[:, dc * 512 + st * 128:
                                           dc * 512 + (st + 1) * 128],
                                wv[:, dc * D + dh * 512:dc * D + (dh + 1) * 512],
                                start=(dc == 0), stop=(dc == DC - 1))
                        lv = locv[:, st * D + dh * 512:st * D + (dh + 1) * 512]
                        nc.vector.tensor_scalar_mul(lv, ps[:], SC_V)
                        nc.sync.dma_start(
                            v_cc_in[:].bitcast(fp8e3)
                            .rearrange("(s two) c -> s two c", two=2)
                            [st * 128:(st + 1) * 128, dh, :],
                            lv)
                nc.gpsimd.collective_compute(
                    "AllGather", mybir.AluOpType.bypass, replica_groups=rg,
                    ins=[v_cc_in[:]], outs=[v_cc_out[:]])

                # Q-projection -> 16*Q -> e4m3 hi/lo packed into qthl
                for po in range(DC):
                    ps = ppsum.tile([128, 512], fp32, tag="pp")
                    for dc in range(DC):
                        nc.tensor.matmul(
                            ps[:], wq[:, dc * D + po * 128:dc * D + (po + 1) * 128],
                            xq[:, dc * 512:(dc + 1) * 512],
                            start=(dc == 0), stop=(dc == DC - 1))
                    qh = qthl[:, po * 1024:po * 1024 + 512]
                    ql = qthl[:, po * 1024 + 512:(po + 1) * 1024]
                    nc.vector.tensor_scalar_mul(qh, ps[:], SC_Q)
                    nc.vector.scalar_tensor_tensor(ql, ps[:], SC_Q, qh, MUL, SUB)

            # ---------------- phase 2: pull gathered K/V into SBUF ----------
            for r in range(NC_N):
                eng = nc.sync if r % 2 == 0 else nc.scalar
                eng.dma_start(
                    ktf[r][:].rearrange("p (a j) -> p a j", a=DC),
                    kt_cc_out[r].rearrange("(a p) j -> p a j", p=128))
            for r in range(NC_N):
                nc.sync.dma_start(
                    vf[r][:].rearrange("p (a b j) -> p a b j", a=4, b=2),
                    v_cc_out[r].bitcast(fp8e3)
                    .rearrange("(a p two) j -> p a two j", p=128, two=2))

            # ---------------- phase 3: scores transposed + exp --------------
            with (
                tc.tile_pool(name="spsum", bufs=4, space="PSUM") as spsum,
                tc.tile_pool(name="opsum", bufs=3, space="PSUM") as opsum,
                tc.tile_pool(name="dpsum", bufs=1, space="PSUM") as dpsum,
                tc.tile_pool(name="obuf", bufs=2) as opool,
            ):
                for kc in range(NKC):
                    r, sl = kc // 4, kc % 4
                    qoff = (kc // 8) * 128
                    w = 512 - qoff
                    ps = spsum.tile([128, 512], fp32, tag="s")
                    for dc in range(DC):
                        lhsT = (ktf[r][:, dc * 512 + sl * 128:
                                       dc * 512 + (sl + 1) * 128]
                                .unsqueeze(1).broadcast_to([128, 2, 128]))
                        rhs = (qthl[:, dc * 1024:(dc + 1) * 1024]
                               .rearrange("p (a j) -> p a j", a=2)[:, :, qoff:])
                        nc.tensor.matmul(ps[:, 0:w], lhsT, rhs, perf_mode=DR,
                                         start=(dc == 0), stop=(dc == DC - 1))
                    nc.vector.tensor_add(
                        ps[:, 0:128], ps[:, 0:128],
                        maskt_sb[:, (kc % 8) * 128:(kc % 8 + 1) * 128])
                    nc.scalar.activation(
                        pt[:, kc * 512 + qoff:(kc + 1) * 512], ps[:, 0:w],
                        mybir.ActivationFunctionType.Exp,
                        bias=0.0, scale=float(act_scale))

                # ---------------- phase 4: P @ V + denominator --------------
                for qt in range(NQT):
                    nkc = 8 * (qt + 1)
                    pso = [opsum.tile([128, 512], fp32, tag="po",
                                      name=f"pso{qt}_{dh}") for dh in range(2)]
                    pden = dpsum.tile([128, 1], fp32, tag="d",
                                      name=f"pden{qt}")
                    for kc in range(nkc):
                        r, sl = kc // 4, kc % 4
                        lhsT = pt[:, kc * 512 + qt * 128:
                                  kc * 512 + (qt + 1) * 128]
                        for dh in range(2):
                            nc.tensor.matmul(
                                pso[dh][:], lhsT,
                                vf[r][:, sl * 1024 + dh * 512:
                                      sl * 1024 + (dh + 1) * 512],
                                start=(kc == 0), stop=(kc == nkc - 1))
                        nc.tensor.matmul(
                            pden[:], lhsT, ones[:],
                            start=(kc == 0), stop=(kc == nkc - 1))
                    den2 = stpool.tile([128, 1], fp32, tag="den")
                    recip = stpool.tile([128, 1], fp32, tag="recip")
                    nc.vector.tensor_scalar_mul(den2[:], pden[:], SC_V)
                    nc.vector.reciprocal(recip[:], den2[:])
                    o_sb = opool.tile([128, D], bf16, tag="o")
                    for dh in range(2):
                        nc.vector.tensor_scalar_mul(
                            o_sb[:, dh * 512:(dh + 1) * 512], pso[dh][:],
                            recip[:])
                    nc.sync.dma_start(out[qt * 128:(qt + 1) * 128, :], o_sb[:])

    nc.compile()
    return nc


def _get_nc():
    if "nc" not in _CACHE:
        _CACHE["nc"] = _build()
    return _CACHE["nc"]


def make_in_maps(x, Wq, Wk, Wv):
    x_bf = np.ascontiguousarray(x).astype(BF16)
    wqt = np.ascontiguousarray(Wq.astype(BF16).T)
    wkt = np.ascontiguousarray(Wk.astype(BF16).T)
    wvt = np.ascontiguousarray(Wv.astype(BF16).T)
    in_maps = []
    for c in range(NC_N):
        xt_q = np.ascontiguousarray(x_bf[c::NC_N].T)
        xt_kv = np.ascontiguousarray(x_bf[c * KVROWS:(c + 1) * KVROWS].T)
        # maskt[p, 128a + i] = 0 if q >= k within the diagonal 1024-band:
        # q row i of a q-tile (global q = c + 8i + 1024qt), k row p of diag
        # chunk a (global k = 128a + p + 1024qt).
        p = np.arange(128)[:, None, None]
        a = np.arange(8)[None, :, None]
        i = np.arange(128)[None, None, :]
        keep = (c + 8 * i) >= (128 * a + p)
        maskt = np.where(keep, 0.0, MASKNEG).astype(np.float32).reshape(128, 1024)
        in_maps.append({"xt_q": xt_q, "xt_kv": xt_kv, "wqt": wqt,
                        "wkt": wkt, "wvt": wvt, "maskt": maskt})
    return in_maps


def run(in_maps, trace=False, tmpdir=None, trace_cores=None):
    from concourse.bass_utils import run_bass_kernel_spmd
    nc = _get_nc()
    return run_bass_kernel_spmd(nc, in_maps, core_ids=list(range(NC_N)),
                                trace=trace, tmpdir=tmpdir,
                                trace_cores=trace_cores)


def kernel(x, Wq, Wk, Wv):
    res = run(make_in_maps(np.asarray(x), np.asarray(Wq),
                           np.asarray(Wk), np.asarray(Wv)))
    full = np.empty((S, D), np.float32)
    for c in range(NC_N):
        full[c::NC_N] = res.results[c]["out"].astype(np.float32)
    return full
